# revision 18
# baseline (speedup 1.0000x reference)
"""Grouped per-channel Linear + ReLU on 8 TRN2 NeuronCores.

Problem: out[b,c,e] = relu(sum_s x[b,s,c] * W[c,s,e] + bias[c,e])
  x: (256, 2048, 32) f32, W: (32, 2048, 2048) f32, bias: (32, 2048) f32
  out: (256, 32, 2048) f32

Sharding: expert/channel parallel - core i computes channels [4i, 4i+4).
Each core runs 4 independent GEMMs of (256x2048)@(2048x2048) with the
contraction dim S on SBUF partitions; x is host-transposed to
[CPC, P, KT, B] fp16 so DMA descriptors are >=2 KB contiguous runs.

Quantization (rel l2 ~1.8e-2, gate 2e-2, matches numpy sim to <1%):
  - k-tiles 0-13: W in float8e3 (E3M4, 4 mantissa bits, scaled to max
    15.0) streamed from HBM straight into the PE as the moving operand
    of an fp16(x) x fp8e3(W) matmul - no on-chip dequant at all, and
    fp8 at normal rate costs the same N cycles/matmul as fp16.
  - k-tiles 14-15: both operands float8e4 with perf_mode=DoubleRow -
    one matmul contracts 256 rows in the same 216 ns, saving ~7us of
    PE time per core. Scales: x/s_x (s_x=|x|max/224) and W*s_x/s_w so
    the product lands in the same 1/s_w units as the main stream.

DMA: channel 0's entire working set rides the sync ring as ONE FIFO in
strict need-order (W k0, x k0-1, W k1, ...). Some cores (notably core
0) see only ~150-250 GB/s early while the fabric saturates at ~435
GB/s; a two-ring split wastes half of whatever a core gets on
not-yet-needed bytes via the SDMA per-packet round-robin, and FIFO
order makes the delivery bandwidth-adaptive. The scalar ring carries
only bias early; later channels prefetch W on sync (at ch k0) and x on
scalar (at ch k8). Output tiles ride scalar, the last channel's
batch-tile-0 stores ride the then-idle sync ring.

HAM: 9 throwaway FULL K=128 N=512 warmup matmuls (K=1 matmuls do NOT
register as PE-busy for the clock gate - measured) bridge ~6.8us to
first-data (~10-12us) so real matmuls start at 2.4 GHz.

bias (pre-divided by s_w) joins the PSUM accumulation as a K=1 matmul
of ones[1,128] x biasq[1,512] after k13, grouped x4 before the DR pair
so LDWEIGHTS doesn't thrash between stationaries (interleaving
measured +0.5us/pair). Eviction is split: VectorE evicts batch-tile 0
with a fused tensor_scalar max(acc*s_w, 0), ScalarE evicts batch-tile
1 with activation Relu(scale=s_w). Channel 0 runs k-major (the DMA
ramp then only sustains one k-tile per 1.73us); middle channels run
bt-major so batch-tile 0's PSUM banks close and evict mid-channel and
the next channel's start=True matmuls never wait on evictions (an
eviction-wait transition stalls ~2.5us and can re-throttle HAM); the
last channel runs per-PSUM-bank so the final eviction+store exposure
is ~1 bank. Outputs leave as fp16.

Measured (8 axon-tunneled TRN2 cores, all at 2.4 GHz): 135.2us max /
133.3us mean HW exec, rel l2 1.78e-2, vs the 164.3us int8 baseline.
"""

import os
import sys

for _p in ("/opt/trn_rl_repo", "/root/.axon_site/_ro/trn_rl_repo"):
    if os.path.isdir(_p) and _p not in sys.path:
        sys.path.insert(0, _p)

import numpy as np
import ml_dtypes

import concourse.bacc as bacc
import concourse.mybir as mybir
from concourse import tile
from concourse.bass_utils import run_bass_kernel_spmd

B, S, C, E = 256, 2048, 32, 2048
NCORES = 8
CPC = C // NCORES          # channels per core = 4
P = 128
KT = S // P                # 16 k-tiles
KTN = KT - 2               # 14 k-tiles on the normal fp16 x fp8e3 path
NBT = B // P               # 2 batch tiles
FREE = 512                 # matmul moving free dim (one PSUM bank of f32)
NET = E // FREE            # 4 e-tiles
FP8_MAX = 15.0             # e3m4 scale target (max normal 15.5)
X8_MAX = 224.0             # e4m3 scale target (TRN max normal 240)
NWARM = 9                  # HAM warmup matmuls: full K=128 N=512 (K=1 MMs
                           # do NOT register as PE-busy for the clock
                           # gate - measured). 9 x 427ns cold bridges
                           # ~6.8us to first-data (~10us)

_nc_cache = {}


def _build(s_w: float):
    nc = bacc.Bacc(None, target_bir_lowering=False)
    xt = nc.dram_tensor("xt", [CPC, P, KT, B], mybir.dt.float16, kind="ExternalInput")
    # W fp8e3, host-layouted [c, partition, ktile, e]: a k-range DMA reads
    # nkt*E contiguous bytes per partition (>=2 KB for nkt>=1).
    w8 = nc.dram_tensor("w8", [CPC, P, KTN, E], mybir.dt.float8e3, kind="ExternalInput")
    # DoubleRow pair (k-tiles 14-15), both operands e4m3
    w4 = nc.dram_tensor("w4", [CPC, P, 2, E], mybir.dt.float8e4, kind="ExternalInput")
    x4 = nc.dram_tensor("x4", [CPC, P, 2, B], mybir.dt.float8e4, kind="ExternalInput")
    biasq = nc.dram_tensor("biasq", [CPC, E], mybir.dt.float16, kind="ExternalInput")
    out = nc.dram_tensor("out", [B, CPC, E], mybir.dt.float16, kind="ExternalOutput")

    with tile.TileContext(nc) as tc:
        with (
            tc.tile_pool(name="const", bufs=1) as const,
            tc.tile_pool(name="xpool", bufs=2) as xpool,
            tc.tile_pool(name="x4pool", bufs=2) as x4pool,
            tc.tile_pool(name="bqpool", bufs=CPC) as bqpool,
            tc.tile_pool(name="wpool", bufs=3) as wpool,
            tc.tile_pool(name="w4pool", bufs=2) as w4pool,
            tc.tile_pool(name="opool", bufs=4) as opool,
            tc.tile_pool(name="psum", bufs=NBT * NET, space="PSUM") as psum,
        ):
            zbias = const.tile([P, 1], mybir.dt.float32, name="zbias", tag="zb")
            nc.any.memset(zbias[:], 0.0)
            ones = const.tile([1, P], mybir.dt.float16, name="ones", tag="ones")
            nc.any.memset(ones[:], 1.0)
            wrm = const.tile([P, FREE], mybir.dt.float16, name="wrm", tag="wrm")
            nc.any.memset(wrm[:], 1.0)

            # HAM warmup: throwaway FULL K=128 N=512 matmuls keep the PE
            # genuinely busy early so the clock gate is open when the
            # first real matmul lands
            psw = psum.tile([P, FREE], mybir.dt.float32, name="psw", tag="ps")
            for _ in range(NWARM):
                nc.tensor.matmul(psw[:], wrm[:, :P], wrm[:], start=True, stop=True)

            # ---- front-loaded critical DMAs ----
            # SDMA engines round-robin per packet across rings; the
            # latency-critical W stream rides the sync ring ALONE in
            # need-order; x slabs, bias and outputs ride the scalar ring.
            # Channel 0's entire working set rides the sync ring as ONE
            # FIFO in strict need-order (W k0, x k0-1, W k1, ...): some
            # cores (notably core 0) see only ~150 GB/s early, and a
            # two-ring split wastes half of that on not-yet-needed bytes
            # via the per-packet round-robin. FIFO = bandwidth-adaptive
            # prioritization. The scalar ring carries only bias early.
            wsb0 = wpool.tile([P, KTN, E], mybir.dt.float8e3, name="wsb", tag="wsb")
            w4sb0 = w4pool.tile([P, 2, E], mybir.dt.float8e4, name="w4sb", tag="w4sb")
            xsb0 = xpool.tile([P, KT, B], mybir.dt.float16, name="xsb")
            x4sb0 = x4pool.tile([P, 2, B], mybir.dt.float8e4, name="x4sb", tag="x4sb")
            nc.sync.dma_start(wsb0[:, 0:1, :], w8[0, :, 0:1, :])
            nc.sync.dma_start(xsb0[:, 0:2, :], xt[0, :, 0:2, :])
            nc.sync.dma_start(wsb0[:, 1:2, :], w8[0, :, 1:2, :])
            nc.sync.dma_start(wsb0[:, 2:4, :], w8[0, :, 2:4, :])
            nc.sync.dma_start(xsb0[:, 2:4, :], xt[0, :, 2:4, :])
            nc.sync.dma_start(wsb0[:, 4:6, :], w8[0, :, 4:6, :])
            nc.sync.dma_start(xsb0[:, 4:8, :], xt[0, :, 4:8, :])
            nc.sync.dma_start(wsb0[:, 6:8, :], w8[0, :, 6:8, :])
            nc.sync.dma_start(wsb0[:, 8:11, :], w8[0, :, 8:11, :])
            nc.sync.dma_start(xsb0[:, 8:12, :], xt[0, :, 8:12, :])
            nc.sync.dma_start(wsb0[:, 11:14, :], w8[0, :, 11:14, :])
            nc.sync.dma_start(xsb0[:, 12:, :], xt[0, :, 12:, :])
            nc.sync.dma_start(w4sb0[:], w4[0, :, :, :])
            nc.sync.dma_start(x4sb0[:], x4[0, :, :, :])
            bqtiles = []
            for c in range(CPC):
                bq = bqpool.tile([1, E], mybir.dt.float16, name="bq", tag="bq")
                nc.scalar.dma_start(bq[:], biasq[c : c + 1, :])
                bqtiles.append(bq)

            xtiles = {0: xsb0}
            wtiles = {0: (wsb0, w4sb0)}
            x4tiles = {0: x4sb0}

            def prefetch_w(c):
                wsb = wpool.tile([P, KTN, E], mybir.dt.float8e3, name="wsb", tag="wsb")
                for g in range(3):
                    nc.sync.dma_start(
                        wsb[:, g * 4 : (g + 1) * 4, :], w8[c, :, g * 4 : (g + 1) * 4, :]
                    )
                nc.sync.dma_start(wsb[:, 12:, :], w8[c, :, 12:, :])
                w4sb = w4pool.tile([P, 2, E], mybir.dt.float8e4, name="w4sb", tag="w4sb")
                nc.sync.dma_start(w4sb[:], w4[c, :, :, :])
                wtiles[c] = (wsb, w4sb)

            def prefetch_x(c):
                # deferred to k==8 so the 2 MB x slab does not steal early
                # SDMA share from the current channel's critical W pieces
                xsb = xpool.tile([P, KT, B], mybir.dt.float16, name="xsb")
                nc.scalar.dma_start(xsb[:], xt[c, :, :, :])
                xtiles[c] = xsb
                x4sb = x4pool.tile([P, 2, B], mybir.dt.float8e4, name="x4sb", tag="x4sb")
                nc.scalar.dma_start(x4sb[:], x4[c, :, :, :])
                x4tiles[c] = x4sb

            def evict(bt, src, dst):
                # DVE takes batch-tile 0 (fused max(acc*s_w, 0)), ScalarE
                # takes batch-tile 1 (Relu activation, scale=s_w)
                if bt == 0:
                    nc.vector.tensor_scalar(
                        dst,
                        src,
                        s_w,
                        0.0,
                        mybir.AluOpType.mult,
                        mybir.AluOpType.max,
                    )
                else:
                    nc.scalar.activation(
                        dst,
                        src,
                        mybir.ActivationFunctionType.Relu,
                        bias=zbias[:],
                        scale=s_w,
                    )

            def bias_mm(ps, bq, et):
                nc.tensor.matmul(
                    ps,
                    ones[:],
                    bq[:, et * FREE : (et + 1) * FREE],
                    start=False,
                    stop=False,
                )

            def dr_mm(ps, x4sb, w4sb, bt, et):
                # k-tiles 14-15: e4m3 x e4m3 DoubleRow - contracts 256
                # rows in one 216 ns matmul and closes the group
                nc.tensor.matmul(
                    ps,
                    x4sb[:, :, bt * P : (bt + 1) * P],
                    w4sb[:, :, et * FREE : (et + 1) * FREE],
                    start=False,
                    stop=True,
                    perf_mode=mybir.MatmulPerfMode.DoubleRow,
                )

            for c in range(CPC - 1):
                xsb = xtiles[c]
                wsb, w4sb = wtiles[c]
                x4sb = x4tiles[c]
                ps = [
                    [
                        psum.tile([P, FREE], mybir.dt.float32, name="ps", tag="ps")
                        for _ in range(NET)
                    ]
                    for _ in range(NBT)
                ]
                bq = bqtiles[c]
                # ch0 runs k-major (both batch-tiles per k-tile) so the DMA
                # ramp only has to sustain one k-tile per 1.73us; later
                # channels have their W fully prefetched and run bt-major:
                # batch-tile 0's banks close and evict mid-channel, so the
                # NEXT channel's start=True matmuls never wait on evictions
                # (an eviction-wait transition stalls ~2.5us and can
                # re-throttle the HAM clock gate).
                if c == 0:
                    for k in range(KTN):
                        for bt in range(NBT):
                            lhsT = xsb[:, k, bt * P : (bt + 1) * P]
                            for et in range(NET):
                                nc.tensor.matmul(
                                    ps[bt][et][:],
                                    lhsT,
                                    wsb[:, k, et * FREE : (et + 1) * FREE],
                                    start=(k == 0),
                                    stop=False,
                                )
                        if k == 0:
                            prefetch_w(c + 1)
                        if k == 8:
                            prefetch_x(c + 1)
                    # bias x4 then DR x4 per batch-tile: grouped so
                    # LDWEIGHTS doesn't thrash between the ones/x4
                    # stationaries (interleaving measured +0.5us per pair)
                    for bt in range(NBT):
                        for et in range(NET):
                            bias_mm(ps[bt][et][:], bq, et)
                        for et in range(NET):
                            dr_mm(ps[bt][et][:], x4sb, w4sb, bt, et)
                    for bt in range(NBT):
                        ot = opool.tile([P, E], mybir.dt.float16)
                        for et in range(NET):
                            evict(bt, ps[bt][et][:], ot[:, et * FREE : (et + 1) * FREE])
                        nc.scalar.dma_start(out[bt * P : (bt + 1) * P, c, :], ot[:])
                else:
                    for bt in range(NBT):
                        for k in range(KTN):
                            lhsT = xsb[:, k, bt * P : (bt + 1) * P]
                            for et in range(NET):
                                nc.tensor.matmul(
                                    ps[bt][et][:],
                                    lhsT,
                                    wsb[:, k, et * FREE : (et + 1) * FREE],
                                    start=(k == 0),
                                    stop=False,
                                )
                            if bt == 0 and k == 0:
                                prefetch_w(c + 1)
                            if bt == 0 and k == 8:
                                prefetch_x(c + 1)
                        for et in range(NET):
                            bias_mm(ps[bt][et][:], bq, et)
                        for et in range(NET):
                            dr_mm(ps[bt][et][:], x4sb, w4sb, bt, et)
                        ot = opool.tile([P, E], mybir.dt.float16)
                        for et in range(NET):
                            evict(bt, ps[bt][et][:], ot[:, et * FREE : (et + 1) * FREE])
                        nc.scalar.dma_start(out[bt * P : (bt + 1) * P, c, :], ot[:])

            # Last channel runs per-PSUM-bank so banks close (and evict +
            # store) one at a time instead of all 8 at the kernel tail.
            c = CPC - 1
            xsb = xtiles[c]
            wsb, w4sb = wtiles[c]
            x4sb = x4tiles[c]
            bq = bqtiles[c]
            for bt in range(NBT):
                ot = opool.tile([P, E], mybir.dt.float16)
                for et in range(NET):
                    psb = psum.tile([P, FREE], mybir.dt.float32, name="ps", tag="ps")
                    for k in range(KTN):
                        nc.tensor.matmul(
                            psb[:],
                            xsb[:, k, bt * P : (bt + 1) * P],
                            wsb[:, k, et * FREE : (et + 1) * FREE],
                            start=(k == 0),
                            stop=False,
                        )
                    bias_mm(psb[:], bq, et)
                    dr_mm(psb[:], x4sb, w4sb, bt, et)
                    dst = ot[:, et * FREE : (et + 1) * FREE]
                    evict(bt, psb[:], dst)
                    # sync ring is idle by now (W stream done); split the
                    # final stores across both rings to shorten the tail
                    oeng = nc.sync if bt == 0 else nc.scalar
                    oeng.dma_start(
                        out[bt * P : (bt + 1) * P, c, et * FREE : (et + 1) * FREE],
                        dst,
                    )
    nc.compile()
    return nc


def _get_nc(s_w: float):
    key = round(float(s_w), 12)
    if key not in _nc_cache:
        _nc_cache[key] = _build(float(s_w))
    return _nc_cache[key]


def _run(x, W, b, **spmd_kwargs):
    s_w = float(np.abs(W).max() / FP8_MAX)
    s_x = float(np.abs(x).max() / X8_MAX)
    nc = _get_nc(s_w)

    SDR = KTN * P  # first contraction row of the DoubleRow pair
    W8 = (W[:, :SDR, :] * (1.0 / s_w)).astype(ml_dtypes.float8_e3m4)
    W4 = (W[:, SDR:, :] * (s_x / s_w)).astype(ml_dtypes.float8_e4m3)

    in_maps = []
    for i in range(NCORES):
        c0, c1 = i * CPC, (i + 1) * CPC
        # x[:, :, c] -> [CPC, P, KT, B]: s = k*P + p
        xc = x[:, :, c0:c1].transpose(2, 1, 0)  # (CPC, S, B)
        xt_i = np.ascontiguousarray(
            xc.reshape(CPC, KT, P, B).transpose(0, 2, 1, 3).astype(np.float16)
        )
        x4_i = np.ascontiguousarray(
            (xc[:, SDR:, :] * (1.0 / s_x))
            .reshape(CPC, 2, P, B)
            .transpose(0, 2, 1, 3)
            .astype(ml_dtypes.float8_e4m3)
        )
        # [CPC, S', E] -> [CPC, P, kt, E] with s = k*P + p
        w8_i = np.ascontiguousarray(
            W8[c0:c1].reshape(CPC, KTN, P, E).transpose(0, 2, 1, 3)
        )
        w4_i = np.ascontiguousarray(
            W4[c0:c1].reshape(CPC, 2, P, E).transpose(0, 2, 1, 3)
        )
        biasq_i = np.ascontiguousarray((b[c0:c1] / s_w).astype(np.float16))
        in_maps.append(
            {"xt": xt_i, "w8": w8_i, "w4": w4_i, "x4": x4_i, "biasq": biasq_i}
        )

    res = run_bass_kernel_spmd(nc, in_maps, core_ids=list(range(NCORES)), **spmd_kwargs)
    out = np.concatenate(
        [r["out"].astype(np.float32) for r in res.results], axis=1
    )
    return out, res


def kernel(x: np.ndarray, W: np.ndarray, b: np.ndarray) -> np.ndarray:
    out, _ = _run(x, W, b)
    return out
